# revision 27
# baseline (speedup 1.0000x reference)
"""Trainium2 Bass kernel for nn_Eq2to2_58815282152312 (PELICAN Eq2to2 layer).

Transpose-pair packing: off-diagonal pixel (i,j) and its partner (j,i)
share one matmul column. Column (k,i), k in [1,64], i in [0,128):
  input col = [x[:, i, j] ; x[:, j, i]]  (K=128), j = (i+k) % 128
  weights W = [[c3, c4], [c4, c3]]      (M=128)
  psum[s, col]    = c3.x(i,j) + c4.x(j,i)   -> out pixel (i,j)
  psum[64+s, col] = c3.x(j,i) + c4.x(i,j)   -> out pixel (j,i)
One 8192-column matmul pass computes the whole c3/c4 contraction for
both pixels of every pair (k=64 pairs computed twice; identical values).

Broadcast addend A[i,s] + CC[j,s] (and mirrored for the upper half) is
added on the PE as two extra matmuls per chunk against identity
indicators: U^T @ I-broadcast gives U[:, i(col)], V^T @ I-band (an
overlapping AP over a doubled identity) gives V[:, (i+k)%128]. Each
chunk is one 3-matmul PSUM accumulation group (U start=True, V, then x
with stop=True). Act applies Lrelu on [128, 1024] pairs -> bf16;
diagonal pixels are patched on host.

Per core: 1 batch, 16 chunks of 512 cols (k-major: chunk = 4 k-values),
8 psum banks, Act per 2-chunk pair, out DMA per 2 pairs. Only 8128 of
the 8192 columns are transferred (the k=64 block is half redundant);
preloads still cover full banks, x-matmul/act/out are trimmed. The aux
tensor stores [w | A|CC|A]: uT and vT are overlapping 128-col slices.
Weights are pair-grouped (U,U,V,V,x,x) to minimize stationary-operand
swaps. All DMA issued from the sync engine (HWDGE), software-pipelined:
rep r+1's inputs are issued before rep r's outputs; xp/aux
double-buffered by rep parity. Steady state is DMA-bound
(~2.08 MB in + 2.08 MB out bf16 per rep, sim slope 11.79 us).
"""
import sys
import numpy as np

sys.path.insert(0, "/opt/trn_rl_repo")

import ml_dtypes

B, N, C, BASIS = 8, 128, 64, 15
AVG = 49.0
SLOPE = 0.01

NK = 64            # k-values (pair offsets 1..64)
NCOL = NK * N      # 8192 padded columns (SBUF layout)
NCOLD = NCOL - 64  # 8128 transferred columns (k=64 block is half redundant)
NCH = 16           # 512-col matmul chunks (4 k-values each)
NPAIR = 8          # chunk pairs: DVE/Act/out-DMA granularity (1024 cols)
NB = 8             # psum banks (512 f32 cols each)
NPIECE = 4         # xpack in-DMA pieces (2048 cols each)
NOUT = 16          # out-tile slots (1024 cols each)
NODMA = 4          # out DMAs per rep (2048 cols each)
NAUX = 1           # aux input DMAs (packed w | A|CC|A)
NAUXC = 320        # aux columns (bf16): w | [A|CC|A] (uT/vT overlap)

_cache = {}


def _build_bass(act_copy=False, reps=1):
    """act_copy=True swaps Lrelu -> Copy so CoreSim's value executor (which
    lacks Lrelu) can run the kernel for race/numeric checks.
    reps>1 repeats the whole body (re-DMAing inputs, serialized between
    reps) so a wall-clock slope over reps isolates per-exec device time."""
    from concourse import bass, mybir

    f32 = mybir.dt.float32
    bf16 = mybir.dt.bfloat16
    act_fn = (
        mybir.ActivationFunctionType.Copy
        if act_copy
        else mybir.ActivationFunctionType.Lrelu
    )

    nc = bass.Bass()
    xi_d = nc.dram_tensor("xi", [128, NCOLD], bf16, kind="ExternalInput")
    aux_d = nc.dram_tensor("aux", [128, NAUXC], bf16, kind="ExternalInput")
    ind_d = nc.dram_tensor("ind2", [128, 192], bf16, kind="ExternalInput")
    out_d = nc.dram_tensor("outT", [128, NCOLD], bf16, kind="ExternalOutput")

    from contextlib import ExitStack

    with ExitStack() as ctx:
        ec = ctx.enter_context
        xp = ec(nc.sbuf_tensor([128, 2 * NCOL], bf16))
        aux = ec(nc.sbuf_tensor([128, 2 * NAUXC], bf16))
        ind2 = ec(nc.sbuf_tensor([128, 192], bf16))
        ot = ec(nc.sbuf_tensor([128, NOUT * 1024], bf16))
        wrm = ec(nc.sbuf_tensor([1, 8], bf16))
        ps = ec(nc.psum_tensor([128, NB * 512], f32))
        s_in = ec(nc.semaphore("s_in"))
        s_ic = ec(nc.semaphore("s_ic"))
        s_x = [ec(nc.semaphore(f"s_x{i}")) for i in range(NPIECE)]
        s_mm = ec(nc.semaphore("s_mm"))
        s_act = ec(nc.semaphore("s_act"))
        s_out = [ec(nc.semaphore(f"s_out{i}")) for i in range(NODMA)]
        block = ec(nc.Block())
        def auxv(par):
            o = par * NAUXC
            return (
                aux[:, o : o + 128],          # wt
                aux[:, o + 128 : o + 256],    # utT = [A | CC]
                aux[:, o + 192 : o + 320],    # vtT = [CC | A] (overlapping)
            )

        @block.sync
        def _(sync):
            # 2048-wide pieces (1984 tail) so chunk c's x-cols always lie
            # within piece c//4
            pofs = [min(2048 * p, NCOLD) for p in range(NPIECE + 1)]
            sync.dma_start(out=ind2[:], in_=ind_d[:]).then_inc(s_ic, 16)
            for it in range(reps + 1):
                if it < reps:
                    r = it
                    par = r % 2
                    if r > 1:
                        # PE done with rep r-2's use of this parity
                        sync.wait_ge(s_mm, NCH * (r - 1))
                    if r > 0:
                        # order same-sem DMA completions across reps
                        sync.wait_ge(s_in, 16 * NAUX * r)
                    sync.dma_start(
                        out=aux[:, par * NAUXC : (par + 1) * NAUXC],
                        in_=aux_d[:],
                    ).then_inc(s_in, 16)
                    xb = par * NCOL
                    for p in range(NPIECE):
                        if r > 0:
                            sync.wait_ge(s_x[p], 16 * r)
                        sync.dma_start(
                            out=xp[:, xb + pofs[p] : xb + pofs[p + 1]],
                            in_=xi_d[:, pofs[p] : pofs[p + 1]],
                        ).then_inc(s_x[p], 16)
                if it >= 1:
                    q = it - 1
                    for d in range(NODMA):
                        D = NODMA * q + d
                        so = (2 * D) % NOUT
                        w_ = min(2048, NCOLD - d * 2048)
                        sync.wait_ge(s_act, NPAIR * q + 2 * d + 2)
                        if q > 0:
                            sync.wait_ge(s_out[d], 16 * q)
                        sync.dma_start(
                            out=out_d[:, d * 2048 : d * 2048 + w_],
                            in_=ot[:, so * 1024 : so * 1024 + w_],
                        ).then_inc(s_out[d], 16)

        @block.tensor
        def _(tensor):
            u_rhs = ind2[:, 0:128].unsqueeze(1).broadcast_to([128, 4, 128])
            tensor.wait_ge(s_ic, 16)
            for r in range(reps):
                par = r % 2
                wt, utT, vtT = auxv(par)
                xb = par * NCOL
                tensor.wait_ge(s_in, 16 * NAUX * (r + 1))
                # preloads only need aux/ind2 (not x pieces); the padding
                # cols of the tail chunk's full-bank preload are never read
                # by act / out-DMA. Chunks 0-7 (banks 0-7) preload up-front
                # so PE stays busy while input pieces stream in; chunks 8-15
                # preload just-in-time (their banks free as same-rep acts
                # retire). Same-weight matmuls stay grouped.

                def u_mm(c):
                    m = NCH * r + c
                    if m >= NB:
                        # bank reuse: act of the pair of chunk m-NB done
                        tensor.wait_ge(s_act, (m - NB) // 2 + 1)
                    tensor.matmul(
                        ps[:, c % NB * 512 : (c % NB + 1) * 512],
                        utT, u_rhs, start=True, stop=False,
                        skip_group_check=(c == NCH - 1),
                    )

                def v_mm(c):
                    k0 = 4 * c
                    vbase = ind2[:, k0 + 1 : k0 + 2]
                    v_rhs = bass.AP(
                        vbase.tensor,
                        vbase.offset,
                        [list(vbase.ap[0]), [1, 4], [1, 128]],
                    )
                    tensor.matmul(
                        ps[:, c % NB * 512 : (c % NB + 1) * 512],
                        vtT, v_rhs, start=False, stop=False,
                        skip_group_check=(c == NCH - 1),
                    )

                for c in range(NB):
                    u_mm(c)
                for c in range(NB):
                    v_mm(c)
                for p in range(NPAIR):
                    chunks = range(2 * p, 2 * p + 2)
                    if p >= NPAIR // 2:
                        for c in chunks:
                            u_mm(c)
                        for c in chunks:
                            v_mm(c)
                    tensor.wait_ge(s_x[p // 2], 16 * (r + 1))
                    for c in chunks:
                        w_ = min(512, NCOLD - c * 512)
                        tensor.matmul(
                            ps[:, c % NB * 512 : c % NB * 512 + w_],
                            wt[:, :],
                            xp[:, xb + c * 512 : xb + c * 512 + w_],
                            start=False,
                            stop=True,
                            skip_group_check=(w_ != 512),
                        ).then_inc(s_mm, 1)

        @block.scalar
        def _(scalar):
            kwargs = {} if act_copy else {"alpha": SLOPE}
            # dummy activation on a loaded SBUF tile: pulls the Lrelu table
            # load off the critical path during the input-DMA lead-in
            scalar.wait_ge(s_in, 16)
            scalar.activation(wrm[:, :], aux[0:1, 0:8], act_fn, **kwargs)
            for r in range(reps):
                for p in range(NPAIR):
                    g = NPAIR * r + p
                    scalar.wait_ge(s_mm, NCH * r + 2 * p + 2)
                    if g >= NOUT:
                        # out-tile slot reuse: covering DMA 2 reps back done
                        scalar.wait_ge(s_out[(g // 2) % NODMA], 16 * (r - 1))
                    so = g % NOUT
                    pb = p % 4 * 1024
                    aw = min(1024, NCOLD - p * 1024)
                    scalar.activation(
                        ot[:, so * 1024 : so * 1024 + aw],
                        ps[:, pb : pb + aw],
                        act_fn,
                        **kwargs,
                    ).then_inc(s_act, 1)

    return nc


def _get_nc(reps=1):
    key = ("nc", reps)
    if key not in _cache:
        _cache[key] = _build_bass(reps=reps)
    return _cache[key]


def _pair_idx():
    i = np.arange(N)
    k = np.arange(1, NK + 1)
    J = (i[None, :] + k[:, None]) % N      # [NK, N]
    I = np.broadcast_to(i[None, :], (NK, N))
    return I, J


def _prep(inputs_arr, coefs00, coefs01, coefs10, coefs11, bias, diag_bias):
    """Host prep: per-batch aux tensors + device input maps."""
    coefs = (
        coefs00[:, None, :] * coefs10[:, :, None]
        + coefs01[None, :, :] * coefs11[:, :, None]
    )  # [d, s, 15]
    c = [np.ascontiguousarray(coefs[:, :, b]) for b in range(BASIS)]

    x_cf = np.ascontiguousarray(inputs_arr.transpose(0, 3, 1, 2))  # [B,d,i,j]
    diag = np.ascontiguousarray(np.diagonal(x_cf, axis1=2, axis2=3))  # [B,d,i]
    rowsum = x_cf.sum(3) / AVG
    colsum = x_cf.sum(2) / AVG
    trace = diag.sum(2) / AVG
    allsum = x_cf.sum((2, 3)) / (AVG * AVG)

    def proj(stat, cb):  # [B,d,i] x [d,s] -> [B,i,s]
        return np.einsum("ndi,ds->nis", stat, cb, optimize=True)

    K0 = trace @ c[13] + allsum @ c[14]  # [B, s]
    A = (
        proj(diag, c[1]) + proj(rowsum, c[9]) + proj(colsum, c[11])
        + K0[:, None, :] + bias[None, None, :]
    )  # [B, i, s]
    CC = proj(diag, c[2]) + proj(rowsum, c[10]) + proj(colsum, c[12])  # [B,j,s]
    K1 = trace @ c[7] + allsum @ c[8]
    E = (
        proj(diag, c[0]) + proj(rowsum, c[5]) + proj(colsum, c[6])
        + K1[:, None, :] + diag_bias[None, None, :]
    )  # [B, i, s]

    # host diagonal values (pre-activation)
    zdiag = proj(diag, c[3] + c[4]) + A + CC + E  # [B, i, s]
    outdiag = np.where(zdiag >= 0, zdiag, SLOPE * zdiag).astype(np.float32)

    bf = ml_dtypes.bfloat16
    w = np.block([[c[3], c[4]], [c[4], c[3]]]).astype(bf)  # [128, 128]
    eye = np.eye(N, dtype=np.float32)
    ind2 = np.concatenate([eye, eye[:, :64]], axis=1)   # [128, 192]
    I, J = _pair_idx()

    in_maps = []
    for n in range(B):
        xlow = x_cf[n][:, I, J].reshape(C, NCOL)[:, :NCOLD]  # x[d, i, j]
        xup = x_cf[n][:, J, I].reshape(C, NCOL)[:, :NCOLD]   # x[d, j, i]
        xi = np.concatenate([xlow, xup], axis=0).astype(bf)  # [128, NCOLD]
        at, cct = A[n].T, CC[n].T                  # [s, i]
        u = np.concatenate([at, cct], axis=0)      # [p, i]
        v = np.concatenate([cct, at], axis=0)      # [p, j]
        at_, cct_ = u.T[:, :64], u.T[:, 64:]   # A[q,s], CC[q,s]
        aux = np.concatenate(
            [w.astype(np.float32), at_, cct_, at_], axis=1
        )
        in_maps.append(
            {"xi": xi, "aux": aux.astype(bf), "ind2": ind2.astype(bf)}
        )
    return in_maps, outdiag


def _gather(results, outdiag, mask):
    out = np.empty((B, N, N, C), np.float32)
    I, J = _pair_idx()
    If = I.reshape(-1)[:NCOLD]
    Jf = J.reshape(-1)[:NCOLD]
    idx = np.arange(N)
    for n in range(B):
        ot = np.asarray(results[n]["outT"]).astype(np.float32)
        out[n][If, Jf, :] = ot[0:C].T       # pixel (i, j)
        out[n][Jf, If, :] = ot[C:128].T     # pixel (j, i)
        out[n][idx, idx, :] = outdiag[n]
    return out * mask


def run_device(in_maps, trace=False):
    from concourse.bass_utils import run_bass_kernel_spmd

    nc = _get_nc()
    return run_bass_kernel_spmd(nc, in_maps, list(range(B)), trace=trace)


def kernel(
    inputs, mask, nobj, coefs00, coefs01, coefs10, coefs11, bias, diag_bias
):
    inputs = np.asarray(inputs, np.float32)
    mask = np.asarray(mask, np.float32)
    in_maps, outdiag = _prep(
        inputs,
        np.asarray(coefs00, np.float32),
        np.asarray(coefs01, np.float32),
        np.asarray(coefs10, np.float32),
        np.asarray(coefs11, np.float32),
        np.asarray(bias, np.float32),
        np.asarray(diag_bias, np.float32),
    )
    res = run_device(in_maps, trace=False)
    return _gather(res.results, outdiag, mask)
